# revision 19
# baseline (speedup 1.0000x reference)
"""DPQNetwork (vq_codebook) Trainium2 kernel.

Math (per reference):
    response[b, c, k] = sum_d inputs[b, c, d] * centroids[c, k, d]
    mse[b, c]   = max_k response
    codes[b, c] = argmax_k response + 64 * c      (first-index tie-break)
    returns (codes int32, mse f32, centroids passthrough)

Shapes: inputs [32768, 64, 16] f32, centroids [64, 64, 16] f32.

Strategy (data-parallel over batch across 8 NeuronCores, 4096 batches/core):
  - Host pre-transposes each x shard to [C, D, Bc] so the device DMA loads
    contraction-major tiles [128p=(8c x 16d), 512b] with 2KB/partition
    contiguous reads.
  - Host builds a block-diagonal weight tensor wblk[8 groups][128, 512]:
    rows (c_sub*16+d), cols (c_sub*64+k) so one full-array fp32 matmul per
    (b-tile, c-group) produces response [128b, 8c*64k] in one PSUM bank.
  - ScalarE stages each 4-bank PSUM tile to SBUF (frees PSUM for the PE
    early); VectorE then does one full-width grouped tensor_reduce(max)
    over k per b-tile -> mse[128, 64], and per bank a max_index
    (first-occurrence search of the 8 per-c maxes in the flat 512-wide
    bank) -> flat idx = 64*c_sub + k. The 512*c_grp column offset is added
    host-side during the unshard (saves a saturated-DVE op). First two x
    chunks are small so the pipeline starts ~10us earlier; x DMAs issue
    from the idle gpsimd queue to parallelize descriptor generation.

Measured on trn2 (8 cores): ~334 us HW exec. The kernel is VectorE-bound:
the grouped max (tensor_reduce, 1 elem/lane/cyc) and the argmax locate
(FIND_INDEX8) must each stream the full 16.7M-element response per core
(~146+180 us, DVE ~98% busy); fp32 matmul (4 cyc/col) is a close second.
"""

import sys
from contextlib import ExitStack

import numpy as np

sys.path.insert(0, "/opt/trn_rl_repo")

from concourse import bacc, mybir, tile  # noqa: E402
from concourse.bass_utils import run_bass_kernel_spmd  # noqa: E402

B, C, K, D = 32768, 64, 64, 16
NCORES = 8
BC = B // NCORES  # 4096 batches per core
BCHUNK = 512      # batches DMA'd per x tile
BTILE = 128       # batches per matmul (PSUM partition dim)
CG = 8            # codebook groups of 8 -> 8*16=128 contraction rows
F32 = mybir.dt.float32
U32 = mybir.dt.uint32

_CACHE = {}


def _build_program():
    nc = bacc.Bacc("TRN2", target_bir_lowering=False, debug=False)
    x_ap = nc.dram_tensor("x_t", [C, D, BC], F32, kind="ExternalInput").ap()
    w_ap = nc.dram_tensor("wblk", [CG, 128, 512], F32, kind="ExternalInput").ap()
    codes_ap = nc.dram_tensor("codes", [BC, C], U32, kind="ExternalOutput").ap()
    mse_ap = nc.dram_tensor("mse", [BC, C], F32, kind="ExternalOutput").ap()

    with tile.TileContext(nc) as tc, ExitStack() as ctx:
        wpool = ctx.enter_context(tc.tile_pool(name="w", bufs=1))
        xpool = ctx.enter_context(tc.tile_pool(name="x", bufs=3))
        opool = ctx.enter_context(tc.tile_pool(name="o", bufs=6))
        pspool = ctx.enter_context(tc.tile_pool(name="ps", bufs=2, space="PSUM"))
        spool = ctx.enter_context(tc.tile_pool(name="stage", bufs=4))

        # Block-diagonal weights, resident for the whole kernel.
        # Split per group so the first matmul isn't gated on the full 2MB.
        wt = wpool.tile([128, CG, 512], F32)
        for g in range(CG):
            nc.sync.dma_start(wt[:, g], w_ap[g])

        # First two chunks are small so the PE/DVE pipeline starts ~10us
        # earlier; steady-state chunks stay at BCHUNK.
        chunks = [(0, 128), (128, 128), (256, 256)] + [
            (o, BCHUNK) for o in range(BCHUNK, BC, BCHUNK)
        ]
        for b_off, b_len in chunks:
            # split low/high groups into separate tiles so h=0 matmuls only
            # wait on the first four DMAs
            xlo = xpool.tile([128, 4, BCHUNK], F32, tag="xlo")
            xhi = xpool.tile([128, 4, BCHUNK], F32, tag="xhi")
            for g in range(CG):
                xt_half = xlo if g < 4 else xhi
                nc.gpsimd.dma_start(
                    xt_half[:, g % 4, :b_len],
                    x_ap[8 * g : 8 * (g + 1), :, b_off : b_off + b_len]
                    .rearrange("c d b -> (c d) b"),
                )
            for bs in range(b_len // BTILE):
                mse_t = opool.tile([128, C], F32, tag="mse")
                codes_t = opool.tile([128, C], U32, tag="codes")
                st = spool.tile([128, CG, 512], F32)
                for h in range(2):  # 4-bank PSUM tiles: c-groups [4h, 4h+4)
                    ps = pspool.tile([128, 4, 512], F32)
                    for q in range(4):
                        g = 4 * h + q
                        nc.tensor.matmul(
                            ps[:, q],
                            lhsT=(xlo if g < 4 else xhi)[:, g % 4, bs * BTILE : (bs + 1) * BTILE],
                            rhs=wt[:, g],
                            start=True,
                            stop=True,
                        )
                    nc.scalar.copy(st[:, 4 * h : 4 * (h + 1)], ps[:])
                # one full-width grouped max over all 64 codebooks
                nc.vector.tensor_reduce(
                    mse_t[:],
                    st[:].rearrange("p q (c k) -> p (q c) k", k=K),
                    axis=mybir.AxisListType.X,
                    op=mybir.AluOpType.max,
                )
                for g in range(CG):
                    nc.vector.max_index(
                        codes_t[:, 8 * g : 8 * (g + 1)],
                        mse_t[:, 8 * g : 8 * (g + 1)],
                        st[:, g],
                    )
                b0 = b_off + bs * BTILE
                nc.sync.dma_start(codes_ap[b0 : b0 + BTILE, :], codes_t[:])
                nc.sync.dma_start(mse_ap[b0 : b0 + BTILE, :], mse_t[:])

    nc.compile()
    return nc


def _get_program():
    if "nc" not in _CACHE:
        _CACHE["nc"] = _build_program()
    return _CACHE["nc"]


def _host_prep(inputs: np.ndarray, centroids: np.ndarray):
    # wblk[g, cs*16+d, cs*64+k] = centroids[8g+cs, k, d]
    cent = np.asarray(centroids, dtype=np.float32).reshape(CG, 8, K, D)
    wblk = np.zeros((CG, 8, D, 8, K), dtype=np.float32)
    for cs in range(8):
        wblk[:, cs, :, cs, :] = cent[:, cs].transpose(0, 2, 1)
    wblk = np.ascontiguousarray(wblk.reshape(CG, 128, 512))

    x = np.asarray(inputs, dtype=np.float32)
    in_maps = []
    for i in range(NCORES):
        shard = x[i * BC : (i + 1) * BC]            # [BC, C, D]
        x_t = np.ascontiguousarray(shard.transpose(1, 2, 0))  # [C, D, BC]
        in_maps.append({"x_t": x_t, "wblk": wblk})
    return in_maps


def run(inputs: np.ndarray, centroids: np.ndarray, trace: bool = False):
    """Run on 8 NeuronCores; returns ((codes, mse, centroids), BassKernelResults)."""
    nc = _get_program()
    in_maps = _host_prep(inputs, centroids)
    res = run_bass_kernel_spmd(nc, in_maps, core_ids=list(range(NCORES)), trace=trace)
    codes = np.concatenate(
        [res.results[i]["codes"].view(np.int32) for i in range(NCORES)], axis=0
    )
    # device emits flat in-bank idx (64*c_sub + k); add 512*c_grp per column
    codes = codes + 512 * (np.arange(C, dtype=np.int32) // 8)
    mse = np.concatenate([res.results[i]["mse"] for i in range(NCORES)], axis=0)
    cent_out = np.asarray(centroids, dtype=np.float32)
    return (codes, mse, cent_out), res


def kernel(inputs: np.ndarray, centroids: np.ndarray):
    out, _ = run(inputs, centroids, trace=False)
    return out


# revision 20
# speedup vs baseline: 1.1998x; 1.1998x over previous
"""DPQNetwork (vq_codebook) Trainium2 kernel.

Math (per reference):
    response[b, c, k] = sum_d inputs[b, c, d] * centroids[c, k, d]
    mse[b, c]   = max_k response
    codes[b, c] = argmax_k response + 64 * c      (first-index tie-break)
    returns (codes int32, mse f32, centroids passthrough)

Shapes: inputs [32768, 64, 16] f32, centroids [64, 64, 16] f32.

Strategy (data-parallel over batch across 8 NeuronCores, 4096 batches/core):
  - Host pre-transposes each x shard to [C, D, Bc] so the device DMA loads
    contraction-major tiles [128p=(8c x 16d), 512b] with 2KB/partition
    contiguous reads.
  - Host builds a block-diagonal weight tensor wblk[8 groups][128, 512]:
    rows (c_sub*16+d), cols (c_sub*64+k) so one full-array fp32 matmul per
    (b-tile, c-group) produces response [128b, 8c*64k] in one PSUM bank.
  - ScalarE stages each 4-bank PSUM tile to SBUF (frees PSUM for the PE
    early); VectorE then does one full-width grouped tensor_reduce(max)
    over k per b-tile -> mse[128, 64], and per bank a max_index
    (first-occurrence search of the 8 per-c maxes in the flat 512-wide
    bank) -> flat idx = 64*c_sub + k. The 512*c_grp column offset is added
    host-side during the unshard (saves a saturated-DVE op). First two x
    chunks are small so the pipeline starts ~10us earlier; x DMAs issue
    from the idle gpsimd queue to parallelize descriptor generation.

Measured on trn2 (8 cores): ~334 us HW exec. The kernel is VectorE-bound:
the grouped max (tensor_reduce, 1 elem/lane/cyc) and the argmax locate
(FIND_INDEX8) must each stream the full 16.7M-element response per core
(~146+180 us, DVE ~98% busy); fp32 matmul (4 cyc/col) is a close second.
"""

import sys
from contextlib import ExitStack

import numpy as np

sys.path.insert(0, "/opt/trn_rl_repo")

from concourse import bacc, mybir, tile  # noqa: E402
from concourse.bass_utils import run_bass_kernel_spmd  # noqa: E402

B, C, K, D = 32768, 64, 64, 16
NCORES = 8
BC = B // NCORES  # 4096 batches per core
BCHUNK = 512      # batches DMA'd per x tile
BTILE = 128       # batches per matmul (PSUM partition dim)
CG = 8            # codebook groups of 8 -> 8*16=128 contraction rows
F32 = mybir.dt.float32
U32 = mybir.dt.uint32

_CACHE = {}


def _build_program():
    nc = bacc.Bacc("TRN2", target_bir_lowering=False, debug=False)
    x_ap = nc.dram_tensor("x_t", [C, D, BC], F32, kind="ExternalInput").ap()
    w_ap = nc.dram_tensor("wblk", [CG, 128, 512], F32, kind="ExternalInput").ap()
    codes_ap = nc.dram_tensor("codes", [BC, C], U32, kind="ExternalOutput").ap()
    mse_ap = nc.dram_tensor("mse", [BC, C], F32, kind="ExternalOutput").ap()

    with tile.TileContext(nc) as tc, ExitStack() as ctx:
        wpool = ctx.enter_context(tc.tile_pool(name="w", bufs=1))
        xpool = ctx.enter_context(tc.tile_pool(name="x", bufs=3))
        opool = ctx.enter_context(tc.tile_pool(name="o", bufs=4))
        pspool = ctx.enter_context(tc.tile_pool(name="ps", bufs=2, space="PSUM"))
        spool = ctx.enter_context(tc.tile_pool(name="stage", bufs=3))

        # Block-diagonal weights, resident for the whole kernel.
        # Split per group so the first matmul isn't gated on the full 2MB.
        wt = wpool.tile([128, CG, 512], F32)
        for g in range(CG):
            nc.sync.dma_start(wt[:, g], w_ap[g])

        # First two chunks are small so the PE/DVE pipeline starts ~10us
        # earlier; steady-state chunks stay at BCHUNK.
        chunks = [(0, 128), (128, 128), (256, 256)] + [
            (o, BCHUNK) for o in range(BCHUNK, BC, BCHUNK)
        ]
        for b_off, b_len in chunks:
            xt = xpool.tile([128, CG, BCHUNK], F32)
            for g in range(CG):
                nc.gpsimd.dma_start(
                    xt[:, g, :b_len],
                    x_ap[8 * g : 8 * (g + 1), :, b_off : b_off + b_len]
                    .rearrange("c d b -> (c d) b"),
                )
            for bs in range(b_len // BTILE):
                mse_t = opool.tile([128, C], F32, tag="mse")
                codes_t = opool.tile([128, C], U32, tag="codes")
                st = spool.tile([128, CG, 512], F32)
                for h in range(2):  # 4-bank PSUM tiles: c-groups [4h, 4h+4)
                    ps = pspool.tile([128, 4, 512], F32)
                    for q in range(4):
                        g = 4 * h + q
                        nc.tensor.matmul(
                            ps[:, q],
                            lhsT=xt[:, g, bs * BTILE : (bs + 1) * BTILE],
                            rhs=wt[:, g],
                            start=True,
                            stop=True,
                        )
                    nc.scalar.copy(st[:, 4 * h : 4 * (h + 1)], ps[:])
                # one full-width grouped max over all 64 codebooks
                nc.vector.tensor_reduce(
                    mse_t[:],
                    st[:].rearrange("p q (c k) -> p (q c) k", k=K),
                    axis=mybir.AxisListType.X,
                    op=mybir.AluOpType.max,
                )
                for g in range(CG):
                    nc.vector.max_index(
                        codes_t[:, 8 * g : 8 * (g + 1)],
                        mse_t[:, 8 * g : 8 * (g + 1)],
                        st[:, g],
                    )
                b0 = b_off + bs * BTILE
                nc.sync.dma_start(codes_ap[b0 : b0 + BTILE, :], codes_t[:])
                nc.sync.dma_start(mse_ap[b0 : b0 + BTILE, :], mse_t[:])

    nc.compile()
    return nc


def _get_program():
    if "nc" not in _CACHE:
        _CACHE["nc"] = _build_program()
    return _CACHE["nc"]


def _host_prep(inputs: np.ndarray, centroids: np.ndarray):
    # wblk[g, cs*16+d, cs*64+k] = centroids[8g+cs, k, d]
    cent = np.asarray(centroids, dtype=np.float32).reshape(CG, 8, K, D)
    wblk = np.zeros((CG, 8, D, 8, K), dtype=np.float32)
    for cs in range(8):
        wblk[:, cs, :, cs, :] = cent[:, cs].transpose(0, 2, 1)
    wblk = np.ascontiguousarray(wblk.reshape(CG, 128, 512))

    x = np.asarray(inputs, dtype=np.float32)
    in_maps = []
    for i in range(NCORES):
        shard = x[i * BC : (i + 1) * BC]            # [BC, C, D]
        x_t = np.ascontiguousarray(shard.transpose(1, 2, 0))  # [C, D, BC]
        in_maps.append({"x_t": x_t, "wblk": wblk})
    return in_maps


def run(inputs: np.ndarray, centroids: np.ndarray, trace: bool = False):
    """Run on 8 NeuronCores; returns ((codes, mse, centroids), BassKernelResults)."""
    nc = _get_program()
    in_maps = _host_prep(inputs, centroids)
    res = run_bass_kernel_spmd(nc, in_maps, core_ids=list(range(NCORES)), trace=trace)
    codes = np.concatenate(
        [res.results[i]["codes"].view(np.int32) for i in range(NCORES)], axis=0
    )
    # device emits flat in-bank idx (64*c_sub + k); add 512*c_grp per column
    codes = codes + 512 * (np.arange(C, dtype=np.int32) // 8)
    mse = np.concatenate([res.results[i]["mse"] for i in range(NCORES)], axis=0)
    cent_out = np.asarray(centroids, dtype=np.float32)
    return (codes, mse, cent_out), res


def kernel(inputs: np.ndarray, centroids: np.ndarray):
    out, _ = run(inputs, centroids, trace=False)
    return out
